# revision 5
# baseline (speedup 1.0000x reference)
"""LSTM warmup + autoregressive decode kernel for Trainium2 (Bass/Tile).

Reference computation (per batch row):
  h,c = 0
  for t in range(T):  h,c = LSTMstep(x_t)        # warmup over input seq
  pred0 = h @ Wd + bd
  for d in range(out_steps-1): h,c = LSTMstep(pred_d); pred_{d+1} = h@Wd+bd
  out[b, s, f] = pred_s

Strategy: data-parallel over 8 NeuronCores (B=4096 -> 512/core); the 512-row
shard splits into TWO interleaved 256-row streams (A leads, B's tanh(c) lags
one step into the next Act sequence so the pipeline closes).

Key optimizations over the bf16 per-gate baseline (Act engine ~98% busy was
the bottleneck; PE 88%):
 - All-sigmoid gates: the g-gate tanh is computed as 2*sigmoid(2 z_g) - 1
   (g-gate weight columns pre-scaled x2 on the host), so all four gates of a
   stream activate in ONE wide [128, 2048] Act op reading the 4-bank PSUM z
   tile -- amortizing the ~185ns per-op Act overhead.
 - fp8e4m3 DoubleRow recurrence: h and U/Ud are fp8; each h-matmul packs the
   two 128-unit K-tiles into a single DoubleRow matmul, cutting PE time ~4x
   vs two bf16 K=128 passes.  The warmup x-pass and the pred/output path
   stay bf16 (fp8 there fails the 2e-2 tolerance; validated in numpy).
 - Decode bias via a DoubleRow ones-matmul (K=1 x 2 k-tiles, second zero).
 - tanh(c) stays on Act, split into per-unit-chunk halves to slot into the
   Act schedule; Act sequence per step: [tcB'(t-1) x2, gA, gB, tcA(t) x2].
 - PSUM: 2 streams x 4 banks = all 8 banks.  The x-transpose scratch
   (warmup) and pred accumulators (decode) live in stream B's o-gate bank
   during its dead window (after gB's read, before the next step's matmuls);
   PSUM pending-zero semantics make the multi-use safe.
 - Within a z bank the accumulation group is bracketed once per step: the
   h-independent x/bias passes issue early (first marks the zero region),
   the h-DR passes close the group late once h8 lands, so PE x-work overlaps
   the Act/DVE tail of the previous step.
"""

import sys

for _p in ("/opt/trn_rl_repo", "/root/.axon_site/_ro/trn_rl_repo"):
    if _p not in sys.path:
        sys.path.insert(0, _p)

import numpy as np

import concourse.bacc as bacc
import concourse.mybir as mybir
import concourse.tile as tile
from concourse import bass_utils

F32 = mybir.dt.float32
BF16 = mybir.dt.bfloat16
FP8 = mybir.dt.float8e4
AF = mybir.ActivationFunctionType
ALU = mybir.AluOpType
DR = mybir.MatmulPerfMode.DoubleRow

N_CORES = 8
F = 64          # input/output feature dim
U = 256         # lstm units
U4 = 4 * U      # gate rows
XLOOK = 3       # steps of x-staging lookahead
NXS = 6         # static x tiles
NS = 2          # batch streams per core
SB = 256        # stream batch

# gate order inside the z tile (cols: gate, u-chunk, batch), keras col order
G_I, G_F, G_G, G_O = 0, 1, 2, 3
GCOL = {G_I: 0, G_F: 256, G_G: 512, G_O: 768}
AUXC = G_O * 2 * SB  # stream-B o-gate bank col base (shared scratch window)


def build_program(B, T, out_steps):
    """Single-core SPMD program for a batch shard of size B (=512)."""
    assert B == 512, "tile geometry is hardcoded for a 512-row shard"
    NB = B // 128

    nc = bacc.Bacc("TRN2", target_bir_lowering=False, debug=False, num_devices=1)

    xin = nc.dram_tensor("xin", [B, T, F], F32, kind="ExternalInput").ap()
    wb_d = nc.dram_tensor("wb", [F + 1, U4], BF16, kind="ExternalInput").ap()
    u8_d = nc.dram_tensor("u8", [128, 2 * U4], FP8, kind="ExternalInput").ap()
    ud8_d = nc.dram_tensor("ud8", [128, 2 * U4], FP8, kind="ExternalInput").ap()
    bdec8_d = nc.dram_tensor("bdec8", [1, 2 * U4], FP8, kind="ExternalInput").ap()
    ones8_d = nc.dram_tensor("ones8", [1, 2 * SB], FP8, kind="ExternalInput").ap()
    wdd2_d = nc.dram_tensor("wdd2", [128, 2 * F], BF16, kind="ExternalInput").ap()
    bdrow_d = nc.dram_tensor("bdrow", [1, F], BF16, kind="ExternalInput").ap()
    ones_d = nc.dram_tensor("ones", [1, 128], BF16, kind="ExternalInput").ap()
    ident_d = nc.dram_tensor("ident", [128, 128], F32, kind="ExternalInput").ap()
    yout = nc.dram_tensor("yout", [B, out_steps, F], F32, kind="ExternalOutput").ap()

    xin_c = xin.rearrange("(c p) t f -> p c (t f)", c=4)   # [128, 4, T*F]
    yout_c = yout.rearrange("(c p) s f -> p c (s f)", c=4)  # [128, 4, S*F]

    with tile.TileContext(nc) as tc:
        import contextlib

        with contextlib.ExitStack() as ctx:
            wpool = ctx.enter_context(tc.tile_pool(name="wpool", bufs=1))
            xspool = ctx.enter_context(tc.tile_pool(name="xspool", bufs=1))
            dpool = ctx.enter_context(tc.tile_pool(name="dpool", bufs=16))
            gpool = ctx.enter_context(tc.tile_pool(name="gpool", bufs=2))
            tpool = ctx.enter_context(tc.tile_pool(name="tpool", bufs=2))
            cpool = ctx.enter_context(tc.tile_pool(name="cpool", bufs=2))
            tcpool = ctx.enter_context(tc.tile_pool(name="tcpool", bufs=2))
            h8pool = ctx.enter_context(tc.tile_pool(name="h8pool", bufs=3))
            hbpool = ctx.enter_context(tc.tile_pool(name="hbpool", bufs=3))
            opool = ctx.enter_context(tc.tile_pool(name="opool", bufs=4))
            zpool = ctx.enter_context(tc.tile_pool(name="zpool", bufs=1, space="PSUM"))

            # ---- weights / constants ----
            ident = wpool.tile([128, 128], F32)
            wb = wpool.tile([F + 1, U4], BF16)
            u8 = wpool.tile([128, 2 * U4], FP8)
            ud8 = wpool.tile([128, 2 * U4], FP8)
            bdec8 = wpool.tile([1, 2 * U4], FP8)
            ones8 = wpool.tile([1, 2 * SB], FP8)
            wdd2 = wpool.tile([128, 2 * F], BF16)
            bdrow = wpool.tile([1, F], BF16)
            ones = wpool.tile([1, 128], BF16)

            # static x tiles: rows 0:64 = x_t^T (bf16), row 64 = ones
            xs = [xspool.tile([F + 1, B], BF16, name=f"xs{j}") for j in range(NXS)]
            for j in range(NXS):
                nc.gpsimd.memset(xs[j][F : F + 1, :], 1.0)

            # per-stream z PSUM tiles: [128, 2048] f32 = 4 banks each
            zt = [zpool.tile([128, 4 * 2 * SB], F32, name=f"z{st}")
                  for st in range(NS)]

            # PE warm-up through the startup DMA wait (p-state ramp)
            for j in range(3):
                nc.tensor.matmul(
                    zt[0][:, 0:B], xs[0][F : F + 1, 0:128], xs[0][F : F + 1, :],
                    start=True, stop=True, skip_group_check=True,
                )

            # ---- x staging ----
            def stage_x_dma(t):
                dt_in = dpool.tile([128, NB * F], F32, tag="din", name=f"din{t}")
                nc.sync.dma_start(
                    dt_in[:].rearrange("p (c f) -> p c f", c=NB),
                    xin_c[:, :, F * t : F * (t + 1)],
                )
                return dt_in

            def stage_x_transpose(t, dt_in):
                # transpose into stream-B's o-gate bank dead window
                xp = zt[1][0:F, AUXC : AUXC + B]
                for bc in range(NB):
                    nc.tensor.transpose(
                        xp[:, 128 * bc : 128 * (bc + 1)],
                        dt_in[:, F * bc : F * (bc + 1)], ident[:],
                    )
                nc.vector.tensor_copy(xs[t % NXS][0:F, :], xp[:, :])

            u8_2 = u8.rearrange("p (two m) -> p two m", two=2)
            ud8_2 = ud8.rearrange("p (two m) -> p two m", two=2)
            bdec8_2 = bdec8.rearrange("p (two m) -> p two m", two=2)
            ones8_2 = ones8.rearrange("p (two n) -> p two n", two=2)

            # ---- PE z-block emission ----
            def emit_x_passes(st, x_t, close):
                """h-independent z work: x@W+b (warmup) or bias (decode)."""
                z = zt[st]
                for q in (G_I, G_F, G_G, G_O):
                    for ch in (0, 1):
                        dst = z[:, q * 2 * SB + ch * SB
                                : q * 2 * SB + (ch + 1) * SB]
                        mcol = GCOL[q] + 128 * ch
                        stop = close and ch == 1
                        if x_t is not None:
                            nc.tensor.matmul(
                                dst, wb[:, mcol : mcol + 128],
                                x_t[:, SB * st : SB * (st + 1)],
                                start=(ch == 0), stop=stop,
                                skip_group_check=True,
                            )
                        else:
                            nc.tensor.matmul(
                                dst, bdec8_2[:, :, mcol : mcol + 128],
                                ones8_2[:, :, 0:SB],
                                start=(ch == 0), stop=stop,
                                perf_mode=DR, skip_group_check=True,
                            )

            def emit_h_passes(st, uw2, h8_prev):
                """DoubleRow h@U accumulation closing each gate-bank group."""
                z = zt[st]
                h2 = h8_prev[:].rearrange("p (two n) -> p two n", two=2)
                for q in (G_I, G_F, G_G, G_O):
                    for ch in (0, 1):
                        dst = z[:, q * 2 * SB + ch * SB
                                : q * 2 * SB + (ch + 1) * SB]
                        mcol = GCOL[q] + 128 * ch
                        nc.tensor.matmul(
                            dst, uw2[:, :, mcol : mcol + 128], h2[:],
                            start=False, stop=(ch == 1),
                            perf_mode=DR, skip_group_check=True,
                        )

            # ---- Act / DVE ----
            def gslice(g_t, q):
                return g_t[:, q * 2 * SB : (q + 1) * 2 * SB]

            def emit_gate_act(t, st):
                g_t = gpool.tile([128, 4 * 2 * SB], BF16, tag=f"g{st}",
                                 name=f"g{t}_{st}")
                nc.scalar.activation(g_t[:], zt[st][:], AF.Sigmoid)
                return g_t

            def emit_chain(t, st, g_t, c_prev):
                """c = f*c_prev + i*(2*sg - 1) ; sg = sigmoid(2 z_g)."""
                c_t = cpool.tile([128, 2 * SB], BF16, tag=f"c{st}",
                                 name=f"c{t}_{st}")
                m2 = tpool.tile([128, 2 * SB], BF16, tag=f"m{st}",
                                name=f"m{t}_{st}")
                i_s, f_s = gslice(g_t, G_I), gslice(g_t, G_F)
                sg_s = gslice(g_t, G_G)
                if c_prev is None:
                    nc.vector.tensor_mul(m2[:], i_s, sg_s)
                    nc.vector.scalar_tensor_tensor(
                        c_t[:], m2[:], 2.0, i_s, ALU.mult, ALU.subtract)
                else:
                    fc = tpool.tile([128, 2 * SB], BF16, tag=f"fc{st}",
                                    name=f"fc{t}_{st}")
                    t1 = tpool.tile([128, 2 * SB], BF16, tag=f"t{st}",
                                    name=f"t{t}_{st}")
                    nc.vector.tensor_mul(fc[:], f_s, c_prev[:])
                    nc.vector.tensor_mul(m2[:], i_s, sg_s)
                    nc.vector.scalar_tensor_tensor(
                        t1[:], m2[:], 2.0, i_s, ALU.mult, ALU.subtract)
                    nc.vector.tensor_add(c_t[:], fc[:], t1[:])
                return c_t

            def emit_flush(t, st, g_t, c_t, need_hb):
                """tanh(c) halves on Act, h8 (fp8) on DVE, bf16 h on gpsimd."""
                tcs = []
                for ch in (0, 1):
                    tch = tcpool.tile([128, SB], BF16, tag=f"tc{st}{ch}",
                                      name=f"tc{t}_{st}_{ch}")
                    nc.scalar.activation(
                        tch[:], c_t[:, SB * ch : SB * (ch + 1)], AF.Tanh)
                    tcs.append(tch)
                h8 = h8pool.tile([128, 2 * SB], FP8, tag=f"h8{st}",
                                 name=f"h8{t}_{st}")
                for ch in (0, 1):
                    nc.vector.tensor_mul(
                        h8[:, SB * ch : SB * (ch + 1)],
                        gslice(g_t, G_O)[:, SB * ch : SB * (ch + 1)],
                        tcs[ch][:],
                    )
                hb = None
                if need_hb:
                    hb = hbpool.tile([128, 2 * SB], BF16, tag=f"hb{st}",
                                     name=f"hb{t}_{st}")
                    for ch in (0, 1):
                        nc.gpsimd.tensor_mul(
                            hb[:, SB * ch : SB * (ch + 1)],
                            gslice(g_t, G_O)[:, SB * ch : SB * (ch + 1)],
                            tcs[ch][:],
                        )
                return h8, hb

            # ---- pred + output (into stream-B o-gate bank dead window) ----
            def emit_pred_mm(st, h_t):
                pp = zt[1][:, AUXC + 2 * F * st : AUXC + 2 * F * (st + 1)]
                for j in range(2):
                    dst = pp[:, F * j : F * (j + 1)]
                    nc.tensor.matmul(
                        dst, ones[0:1, 0:128], bdrow[:], start=True,
                        stop=False, skip_group_check=True,
                    )
                    nc.tensor.matmul(
                        dst, h_t[:, 128 * j : 128 * (j + 1)], wdd2[:, 0:F],
                        start=False, stop=False, skip_group_check=True,
                    )
                    nc.tensor.matmul(
                        dst, h_t[:, SB + 128 * j : SB + 128 * (j + 1)],
                        wdd2[:, F : 2 * F],
                        start=False, stop=True, skip_group_check=True,
                    )
                return pp

            def emit_pred_out(s, st, pp):
                osb = opool.tile([128, 2 * F], F32, tag="ot", name=f"o{s}_{st}")
                nc.vector.tensor_copy(osb[:], pp[:])
                nc.sync.dma_start(
                    yout_c[:, 2 * st : 2 * st + 2, F * s : F * (s + 1)],
                    osb[:].rearrange("p (c f) -> p c f", c=2),
                )

            # ---- prologue ----
            dma_q = {}
            dma_q[0] = stage_x_dma(0)
            nc.sync.dma_start(ident[:], ident_d[:])
            nc.sync.dma_start(wb[:], wb_d[:])
            for t in range(1, min(XLOOK, T)):
                dma_q[t] = stage_x_dma(t)
            nc.sync.dma_start(u8[:], u8_d[:])
            nc.sync.dma_start(ones8[:], ones8_d[:])
            nc.sync.dma_start(ones[:], ones_d[:])
            nc.sync.dma_start(ud8[:], ud8_d[:])
            nc.sync.dma_start(wdd2[:], wdd2_d[:])
            nc.sync.dma_start(bdec8[:], bdec8_d[:])
            nc.sync.dma_start(bdrow[:], bdrow_d[:])
            for t in range(min(XLOOK, T)):
                stage_x_transpose(t, dma_q.pop(t))
            if XLOOK < T:
                dma_q[XLOOK] = stage_x_dma(XLOOK)

            n_steps = T + (out_steps - 1)
            h8_prev = [None] * NS
            c_prev = [None] * NS
            pend_b = None  # (t, g_t, c_t) stream-B flush deferred to step t+1
            hbs = {}

            for t in range(n_steps):
                warm = t < T
                x_t = xs[t % NXS] if warm else None
                uw2 = u8_2 if warm else ud8_2
                first = t == 0
                need_hb = t >= T - 1

                # 1. lagged stream-B flush of step t-1
                if pend_b is not None:
                    tb, gb_t, cb_t = pend_b
                    h8_prev[1], hb = emit_flush(tb, 1, gb_t, cb_t, tb >= T - 1)
                    if hb is not None:
                        hbs.setdefault(tb, [None] * NS)[1] = hb
                    pend_b = None

                # 2./3. per-stream PE z block + merged gate activation
                g_ts = []
                for st in range(NS):
                    emit_x_passes(st, x_t, close=first)
                    if not first:
                        emit_h_passes(st, uw2, h8_prev[st])
                    g_ts.append(emit_gate_act(t, st))

                # 4./5. DVE chains
                c_ts = [emit_chain(t, st, g_ts[st], c_prev[st])
                        for st in range(NS)]
                c_prev = c_ts

                # 6. stream-A flush now; stream-B lags to step t+1
                h8_prev[0], hb = emit_flush(t, 0, g_ts[0], c_ts[0], need_hb)
                if hb is not None:
                    hbs.setdefault(t, [None] * NS)[0] = hb
                pend_b = (t, g_ts[1], c_ts[1])

                # 7. x staging for upcoming steps (stream-B o-bank window)
                if t + 1 < n_steps:
                    if t + XLOOK < T and t + XLOOK in dma_q:
                        stage_x_transpose(t + XLOOK, dma_q.pop(t + XLOOK))
                    if t + XLOOK + 1 < T:
                        dma_q[t + XLOOK + 1] = stage_x_dma(t + XLOOK + 1)

                # 8. pred matmuls for the previous step's h (lag 1)
                if t >= T and (t - 1) in hbs:
                    for st in range(NS):
                        pp = emit_pred_mm(st, hbs[t - 1][st])
                        emit_pred_out(t - T, st, pp)
                    del hbs[t - 1]

            # epilogue: flush stream B's last step, then the final preds
            tb, gb_t, cb_t = pend_b
            h8_prev[1], hb = emit_flush(tb, 1, gb_t, cb_t, True)
            hbs.setdefault(tb, [None] * NS)[1] = hb
            for st in range(NS):
                pp = emit_pred_mm(st, hbs[n_steps - 1][st])
                emit_pred_out(out_steps - 1, st, pp)

    nc.compile()
    return nc


_CACHE = {}


def _get_program(key):
    if key not in _CACHE:
        _CACHE[key] = build_program(*key)
    return _CACHE[key]


def _host_prep(W, Uk, b, Wd, bd):
    bf16 = mybir.dt.np(BF16)
    fp8 = mybir.dt.np(FP8)
    W64 = W.astype(np.float64)
    Ud = (Uk.astype(np.float64) + Wd.astype(np.float64) @ W64).astype(np.float32)
    bdec = (b.astype(np.float64) + bd.astype(np.float64) @ W64).astype(np.float32)

    def scale_g(M):
        M = M.copy()
        M[..., 512:768] *= 2.0  # g-gate cols: tanh(z) = 2*sigmoid(2z) - 1
        return M

    Ws, bs = scale_g(W), scale_g(b)
    Us, Uds, bdecs = scale_g(Uk), scale_g(Ud), scale_g(bdec)
    wb = np.concatenate([Ws, bs.reshape(1, -1)], axis=0)          # [65, 1024]
    u8 = np.concatenate([Us[0:128], Us[128:256]], axis=1)         # [128, 2048]
    ud8 = np.concatenate([Uds[0:128], Uds[128:256]], axis=1)
    bdec8 = np.concatenate(
        [bdecs.reshape(1, -1), np.zeros((1, U4), np.float32)], axis=1)
    wdd2 = np.concatenate([Wd[0:128], Wd[128:256]], axis=1)       # [128, 128]
    return {
        "wb": wb.astype(bf16),
        "u8": u8.astype(fp8),
        "ud8": ud8.astype(fp8),
        "bdec8": bdec8.astype(fp8),
        "ones8": np.ones((1, 2 * SB), dtype=fp8),
        "wdd2": wdd2.astype(bf16),
        "bdrow": bd.reshape(1, -1).astype(bf16),
        "ones": np.ones((1, 128), dtype=bf16),
        "ident": np.eye(128, dtype=np.float32),
    }


def kernel(inputs, W, U, b, Wd, bd, out_steps):
    inputs = np.asarray(inputs, dtype=np.float32)
    W = np.asarray(W, dtype=np.float32)
    U_ = np.asarray(U, dtype=np.float32)
    b_ = np.asarray(b, dtype=np.float32)
    Wd = np.asarray(Wd, dtype=np.float32)
    bd = np.asarray(bd, dtype=np.float32)
    out_steps = int(out_steps)

    B_full, T, _ = inputs.shape
    assert B_full % N_CORES == 0
    Bc = B_full // N_CORES

    nc = _get_program((Bc, T, out_steps))
    shared = _host_prep(W, U_, b_, Wd, bd)
    in_maps = [
        {"xin": np.ascontiguousarray(inputs[i * Bc : (i + 1) * Bc]), **shared}
        for i in range(N_CORES)
    ]
    res = bass_utils.run_bass_kernel_spmd(nc, in_maps, core_ids=list(range(N_CORES)))
    out = np.concatenate([res.results[i]["yout"] for i in range(N_CORES)], axis=0)
    return out


# revision 7
# speedup vs baseline: 1.0108x; 1.0108x over previous
"""LSTM warmup + autoregressive decode kernel for Trainium2 (Bass/Tile).

Reference computation (per batch row):
  h,c = 0
  for t in range(T):  h,c = LSTMstep(x_t)        # warmup over input seq
  pred0 = h @ Wd + bd
  for d in range(out_steps-1): h,c = LSTMstep(pred_d); pred_{d+1} = h@Wd+bd
  out[b, s, f] = pred_s

Strategy: data-parallel over 8 NeuronCores (B=4096 -> 512/core); the 512-row
shard splits into FOUR interleaved 128-row streams so each stream's serial
recurrence tail (tanh(c) -> h8 -> h@U matmuls) hides under the other three
streams' activation work.

Key optimizations over the bf16 per-gate 2-stream baseline (Act engine ~98%
busy was the bottleneck):
 - All-sigmoid gates: the g-gate tanh is computed as 2*sigmoid(2 z_g) - 1
   (g-gate weight columns pre-scaled x2 on the host), so all four gates of a
   stream activate in ONE [128, 1024] Act op reading the 2-bank PSUM z tile,
   amortizing the ~185ns per-op Act overhead.
 - fp8e4m3 DoubleRow recurrence: h and U/Ud are fp8; each h-matmul packs the
   two 128-unit K-tiles into one DoubleRow matmul, ~4x cheaper on PE than
   two bf16 K=128 passes.  The warmup x-pass and pred path stay bf16 (fp8
   there fails the 2e-2 tolerance; validated in numpy).
 - Decode bias via a DoubleRow ones-matmul (K=1 x 2 k-tiles, second zero).
 - tanh(c) on Act, PAIRED across streams (c tiles of streams 0+1 / 2+3 share
   one [128, 512] tile activated in one op) to halve tc op overheads.
 - PSUM: 4 streams x 2 banks = all 8 banks.  x-transpose scratch (warmup)
   and pred accumulators (decode) borrow stream 3's second bank during its
   dead window (after g3's activation read, before the next step's matmuls);
   PSUM pending-zero semantics make the multi-use safe.
 - Within a z bank the accumulation group is bracketed once per step: the
   h-independent x/bias passes issue early (first marks the zero region),
   the h-DR passes close the group once h8 lands, overlapping PE x-work
   with the previous step's Act/DVE tail.
 - Elementwise h work that tolerates latency runs on the idle engines:
   x-staging copy and the bf16 h for the pred path go to GPSIMD.
"""

import sys

for _p in ("/opt/trn_rl_repo", "/root/.axon_site/_ro/trn_rl_repo"):
    if _p not in sys.path:
        sys.path.insert(0, _p)

import numpy as np

import concourse.bacc as bacc
import concourse.mybir as mybir
import concourse.tile as tile
from concourse import bass_utils

F32 = mybir.dt.float32
BF16 = mybir.dt.bfloat16
FP8 = mybir.dt.float8e4
AF = mybir.ActivationFunctionType
ALU = mybir.AluOpType
DR = mybir.MatmulPerfMode.DoubleRow

N_CORES = 8
F = 64          # input/output feature dim
U = 256         # lstm units
U4 = 4 * U      # gate rows
XLOOK = 3       # steps of x-staging lookahead
NXS = 6         # static x tiles
NS = 4          # batch streams per core
SB = 128        # stream batch

# gate order inside a stream's z tile (cols: gate, u-chunk, batch)
G_I, G_F, G_G, G_O = 0, 1, 2, 3
GCOL = {G_I: 0, G_F: 256, G_G: 512, G_O: 768}
ZW = 4 * 2 * SB          # z tile width per stream (1024 f32 = 2 banks)
AUXC = 2 * 2 * SB        # col base of stream-3's second bank (scratch window)


def build_program(B, T, out_steps):
    """Single-core SPMD program for a batch shard of size B (=512)."""
    assert B == 512, "tile geometry is hardcoded for a 512-row shard"
    NB = B // 128

    nc = bacc.Bacc("TRN2", target_bir_lowering=False, debug=False, num_devices=1)

    xin = nc.dram_tensor("xin", [B, T, F], F32, kind="ExternalInput").ap()
    wb_d = nc.dram_tensor("wb", [F + 1, U4], BF16, kind="ExternalInput").ap()
    u8_d = nc.dram_tensor("u8", [128, 2 * U4], FP8, kind="ExternalInput").ap()
    ud8_d = nc.dram_tensor("ud8", [128, 2 * U4], FP8, kind="ExternalInput").ap()
    bdec8_d = nc.dram_tensor("bdec8", [1, 2 * U4], FP8, kind="ExternalInput").ap()
    ones8_d = nc.dram_tensor("ones8", [1, 2 * SB], FP8, kind="ExternalInput").ap()
    wdd2_d = nc.dram_tensor("wdd2", [128, 2 * F], BF16, kind="ExternalInput").ap()
    bdrow_d = nc.dram_tensor("bdrow", [1, F], BF16, kind="ExternalInput").ap()
    ones_d = nc.dram_tensor("ones", [1, 128], BF16, kind="ExternalInput").ap()
    ident_d = nc.dram_tensor("ident", [128, 128], F32, kind="ExternalInput").ap()
    yout = nc.dram_tensor("yout", [B, out_steps, F], F32, kind="ExternalOutput").ap()

    xin_c = xin.rearrange("(c p) t f -> p c (t f)", c=4)   # [128, 4, T*F]
    yout_c = yout.rearrange("(c p) s f -> p c (s f)", c=4)  # [128, 4, S*F]

    with tile.TileContext(nc) as tc:
        import contextlib

        with contextlib.ExitStack() as ctx:
            wpool = ctx.enter_context(tc.tile_pool(name="wpool", bufs=1))
            xspool = ctx.enter_context(tc.tile_pool(name="xspool", bufs=1))
            dpool = ctx.enter_context(tc.tile_pool(name="dpool", bufs=16))
            gpool = ctx.enter_context(tc.tile_pool(name="gpool", bufs=2))
            tpool = ctx.enter_context(tc.tile_pool(name="tpool", bufs=2))
            cpool = ctx.enter_context(tc.tile_pool(name="cpool", bufs=2))
            tcpool = ctx.enter_context(tc.tile_pool(name="tcpool", bufs=2))
            h8pool = ctx.enter_context(tc.tile_pool(name="h8pool", bufs=3))
            hbpool = ctx.enter_context(tc.tile_pool(name="hbpool", bufs=3))
            opool = ctx.enter_context(tc.tile_pool(name="opool", bufs=4))
            zpool = ctx.enter_context(tc.tile_pool(name="zpool", bufs=1, space="PSUM"))

            # ---- weights / constants ----
            ident = wpool.tile([128, 128], F32)
            wb = wpool.tile([F + 1, U4], BF16)
            u8 = wpool.tile([128, 2 * U4], FP8)
            ud8 = wpool.tile([128, 2 * U4], FP8)
            bdec8 = wpool.tile([1, 2 * U4], FP8)
            ones8 = wpool.tile([1, 2 * SB], FP8)
            wdd2 = wpool.tile([128, 2 * F], BF16)
            bdrow = wpool.tile([1, F], BF16)
            ones = wpool.tile([1, 128], BF16)

            # static x tiles: rows 0:64 = x_t^T (bf16), row 64 = ones
            xs = [xspool.tile([F + 1, B], BF16, name=f"xs{j}") for j in range(NXS)]
            for j in range(NXS):
                nc.gpsimd.memset(xs[j][F : F + 1, :], 1.0)

            # per-stream z PSUM tiles: [128, 1024] f32 = 2 banks each
            zt = [zpool.tile([128, ZW], F32, name=f"z{st}") for st in range(NS)]

            # PE warm-up through the startup DMA wait (p-state ramp)
            for j in range(3):
                nc.tensor.matmul(
                    zt[0][:, 0:512], xs[0][F : F + 1, 0:128], xs[0][F : F + 1, 0:512],
                    start=True, stop=True, skip_group_check=True,
                )

            # ---- x staging ----
            def stage_x_dma(t):
                dt_in = dpool.tile([128, NB * F], F32, tag="din", name=f"din{t}")
                nc.sync.dma_start(
                    dt_in[:].rearrange("p (c f) -> p c f", c=NB),
                    xin_c[:, :, F * t : F * (t + 1)],
                )
                return dt_in

            def stage_x_transpose(t, dt_in):
                # transpose into stream-3's second bank dead window
                # (Pool/GPSIMD cannot read PSUM on real HW; copy on DVE)
                xp = zt[3][0:F, AUXC : AUXC + B]
                for bc in range(NB):
                    nc.tensor.transpose(
                        xp[:, 128 * bc : 128 * (bc + 1)],
                        dt_in[:, F * bc : F * (bc + 1)], ident[:],
                    )
                nc.vector.tensor_copy(xs[t % NXS][0:F, :], xp[:, :])

            u8_2 = u8.rearrange("p (two m) -> p two m", two=2)
            ud8_2 = ud8.rearrange("p (two m) -> p two m", two=2)
            bdec8_2 = bdec8.rearrange("p (two m) -> p two m", two=2)
            ones8_2 = ones8.rearrange("p (two n) -> p two n", two=2)

            # ---- PE z-block emission (bank = 2 gates; one group per bank) --
            def emit_x_passes(st, x_t, close):
                """h-independent z work: x@W+b (warmup) or bias (decode)."""
                z = zt[st]
                for q in (G_I, G_F, G_G, G_O):
                    for ch in (0, 1):
                        dst = z[:, q * 2 * SB + ch * SB
                                : q * 2 * SB + (ch + 1) * SB]
                        mcol = GCOL[q] + 128 * ch
                        bank_first = q in (G_I, G_G) and ch == 0
                        bank_last = q in (G_F, G_O) and ch == 1
                        if x_t is not None:
                            nc.tensor.matmul(
                                dst, wb[:, mcol : mcol + 128],
                                x_t[:, SB * st : SB * (st + 1)],
                                start=bank_first, stop=close and bank_last,
                                skip_group_check=True,
                            )
                        else:
                            nc.tensor.matmul(
                                dst, bdec8_2[:, :, mcol : mcol + 128],
                                ones8_2[:, :, 0:SB],
                                start=bank_first, stop=close and bank_last,
                                perf_mode=DR, skip_group_check=True,
                            )

            def emit_h_passes(st, uw2, h8_ap):
                """DoubleRow h@U accumulation closing each gate-bank group."""
                z = zt[st]
                h2 = h8_ap.rearrange("p (two n) -> p two n", two=2)
                for q in (G_I, G_F, G_G, G_O):
                    for ch in (0, 1):
                        dst = z[:, q * 2 * SB + ch * SB
                                : q * 2 * SB + (ch + 1) * SB]
                        mcol = GCOL[q] + 128 * ch
                        bank_last = q in (G_F, G_O) and ch == 1
                        nc.tensor.matmul(
                            dst, uw2[:, :, mcol : mcol + 128], h2[:],
                            start=False, stop=bank_last,
                            perf_mode=DR, skip_group_check=True,
                        )

            # ---- Act / DVE ----
            def gslice(g_t, q):
                return g_t[:, q * 2 * SB : (q + 1) * 2 * SB]

            def emit_gate_act(t, st):
                g_t = gpool.tile([128, ZW], BF16, tag=f"g{st}", name=f"g{t}_{st}")
                nc.scalar.activation(g_t[:], zt[st][:], AF.Sigmoid)
                return g_t

            def emit_chain(t, st, g_t, c_pair, c_prev):
                """c = f*c_prev + i*(2*sg - 1) into half of the shared pair
                tile; sg = sigmoid(2 z_g)."""
                c_t = c_pair[:, 2 * SB * (st % 2) : 2 * SB * (st % 2 + 1)]
                m2 = tpool.tile([128, 2 * SB], BF16, tag=f"m{st}",
                                name=f"m{t}_{st}")
                i_s, f_s = gslice(g_t, G_I), gslice(g_t, G_F)
                sg_s = gslice(g_t, G_G)
                if c_prev is None:
                    nc.vector.tensor_mul(m2[:], i_s, sg_s)
                    nc.vector.scalar_tensor_tensor(
                        c_t, m2[:], 2.0, i_s, ALU.mult, ALU.subtract)
                else:
                    fc = tpool.tile([128, 2 * SB], BF16, tag=f"fc{st}",
                                    name=f"fc{t}_{st}")
                    t1 = tpool.tile([128, 2 * SB], BF16, tag=f"t{st}",
                                    name=f"t{t}_{st}")
                    nc.vector.tensor_mul(fc[:], f_s, c_prev)
                    nc.vector.tensor_mul(m2[:], i_s, sg_s)
                    nc.vector.scalar_tensor_tensor(
                        t1[:], m2[:], 2.0, i_s, ALU.mult, ALU.subtract)
                    nc.vector.tensor_add(c_t, fc[:], t1[:])
                return c_t

            # ---- pred + output (stream-3 second-bank dead window) ----
            def emit_pred_mm(st, h_t):
                pp = zt[3][:, AUXC + 2 * F * st : AUXC + 2 * F * st + F]
                nc.tensor.matmul(
                    pp, ones[0:1, 0:128], bdrow[:], start=True, stop=False,
                    skip_group_check=True,
                )
                nc.tensor.matmul(
                    pp, h_t[:, 0:128], wdd2[:, 0:F],
                    start=False, stop=False, skip_group_check=True,
                )
                nc.tensor.matmul(
                    pp, h_t[:, SB : SB + 128], wdd2[:, F : 2 * F],
                    start=False, stop=True, skip_group_check=True,
                )
                return pp

            def emit_pred_out(s, st, pp):
                osb = opool.tile([128, F], F32, tag="ot", name=f"o{s}_{st}")
                nc.vector.tensor_copy(osb[:], pp)
                nc.sync.dma_start(
                    yout_c[:, st : st + 1, F * s : F * (s + 1)],
                    osb[:].rearrange("p (c f) -> p c f", c=1),
                )

            # ---- prologue ----
            dma_q = {}
            dma_q[0] = stage_x_dma(0)
            nc.sync.dma_start(ident[:], ident_d[:])
            nc.sync.dma_start(wb[:], wb_d[:])
            for t in range(1, min(XLOOK, T)):
                dma_q[t] = stage_x_dma(t)
            nc.sync.dma_start(u8[:], u8_d[:])
            nc.sync.dma_start(ones8[:], ones8_d[:])
            nc.sync.dma_start(ones[:], ones_d[:])
            nc.sync.dma_start(ud8[:], ud8_d[:])
            nc.sync.dma_start(wdd2[:], wdd2_d[:])
            nc.sync.dma_start(bdec8[:], bdec8_d[:])
            nc.sync.dma_start(bdrow[:], bdrow_d[:])
            for t in range(min(XLOOK, T)):
                stage_x_transpose(t, dma_q.pop(t))
            if XLOOK < T:
                dma_q[XLOOK] = stage_x_dma(XLOOK)

            n_steps = T + (out_steps - 1)
            h8_prev = [None] * NS   # (tile, col offset) per stream
            c_prev = [None] * NS
            hbs = {}

            for t in range(n_steps):
                warm = t < T
                x_t = xs[t % NXS] if warm else None
                uw2 = u8_2 if warm else ud8_2
                first = t == 0
                need_hb = t >= T - 1

                # PE: h-independent passes open each bank group; DR h passes
                # close them (waiting on h8 of t-1); then the merged gate act
                g_ts = []
                for st in range(NS):
                    emit_x_passes(st, x_t, close=first)
                    if not first:
                        emit_h_passes(st, uw2, h8_prev[st])
                    g_ts.append(emit_gate_act(t, st))

                # DVE chains into shared pair tiles; paired tanh(c) on Act
                c01 = cpool.tile([128, 4 * SB], BF16, tag="c01", name=f"c01_{t}")
                c23 = cpool.tile([128, 4 * SB], BF16, tag="c23", name=f"c23_{t}")
                pair = {0: c01, 1: c01, 2: c23, 3: c23}
                c_ts = [emit_chain(t, st, g_ts[st], pair[st], c_prev[st])
                        for st in range(NS)]
                c_prev = c_ts

                tc01 = tcpool.tile([128, 4 * SB], BF16, tag="tc01",
                                   name=f"tc01_{t}")
                tc23 = tcpool.tile([128, 4 * SB], BF16, tag="tc23",
                                   name=f"tc23_{t}")
                nc.scalar.activation(tc01[:], c01[:], AF.Tanh)
                nc.scalar.activation(tc23[:], c23[:], AF.Tanh)
                tcp = {0: tc01, 1: tc01, 2: tc23, 3: tc23}

                # h8 (fp8 for the DR recurrence) on DVE; bf16 h on GPSIMD
                h8p01 = h8pool.tile([128, 4 * SB], FP8, tag="h801",
                                    name=f"h801_{t}")
                h8p23 = h8pool.tile([128, 4 * SB], FP8, tag="h823",
                                    name=f"h823_{t}")
                h8p = {0: h8p01, 1: h8p01, 2: h8p23, 3: h8p23}
                for st in range(NS):
                    off = 2 * SB * (st % 2)
                    nc.vector.tensor_mul(
                        h8p[st][:, off : off + 2 * SB], gslice(g_ts[st], G_O),
                        tcp[st][:, off : off + 2 * SB],
                    )
                    h8_prev[st] = h8p[st][:, off : off + 2 * SB]
                if need_hb:
                    hbt = hbpool.tile([128, NS * 2 * SB], BF16, tag="hb",
                                      name=f"hb_{t}")
                    for st in range(NS):
                        off = 2 * SB * (st % 2)
                        nc.gpsimd.tensor_mul(
                            hbt[:, 2 * SB * st : 2 * SB * (st + 1)],
                            gslice(g_ts[st], G_O),
                            tcp[st][:, off : off + 2 * SB],
                        )
                    hbs[t] = hbt

                # x staging for upcoming steps (stream-3 second-bank window)
                if t + 1 < n_steps:
                    if t + XLOOK < T and t + XLOOK in dma_q:
                        stage_x_transpose(t + XLOOK, dma_q.pop(t + XLOOK))
                    if t + XLOOK + 1 < T:
                        dma_q[t + XLOOK + 1] = stage_x_dma(t + XLOOK + 1)

                # pred matmuls for the previous step's h (lag 1)
                if t >= T and (t - 1) in hbs:
                    hbt = hbs.pop(t - 1)
                    for st in range(NS):
                        pp = emit_pred_mm(st, hbt[:, 2 * SB * st : 2 * SB * (st + 1)])
                        emit_pred_out(t - T, st, pp)

            # epilogue: final preds
            hbt = hbs.pop(n_steps - 1)
            for st in range(NS):
                pp = emit_pred_mm(st, hbt[:, 2 * SB * st : 2 * SB * (st + 1)])
                emit_pred_out(out_steps - 1, st, pp)

    nc.compile()
    return nc


_CACHE = {}


def _get_program(key):
    if key not in _CACHE:
        _CACHE[key] = build_program(*key)
    return _CACHE[key]


def _host_prep(W, Uk, b, Wd, bd):
    bf16 = mybir.dt.np(BF16)
    fp8 = mybir.dt.np(FP8)
    W64 = W.astype(np.float64)
    Ud = (Uk.astype(np.float64) + Wd.astype(np.float64) @ W64).astype(np.float32)
    bdec = (b.astype(np.float64) + bd.astype(np.float64) @ W64).astype(np.float32)

    def scale_g(M):
        M = M.copy()
        M[..., 512:768] *= 2.0  # g-gate cols: tanh(z) = 2*sigmoid(2z) - 1
        return M

    Ws, bs = scale_g(W), scale_g(b)
    Us, Uds, bdecs = scale_g(Uk), scale_g(Ud), scale_g(bdec)
    wb = np.concatenate([Ws, bs.reshape(1, -1)], axis=0)          # [65, 1024]
    u8 = np.concatenate([Us[0:128], Us[128:256]], axis=1)         # [128, 2048]
    ud8 = np.concatenate([Uds[0:128], Uds[128:256]], axis=1)
    bdec8 = np.concatenate(
        [bdecs.reshape(1, -1), np.zeros((1, U4), np.float32)], axis=1)
    wdd2 = np.concatenate([Wd[0:128], Wd[128:256]], axis=1)       # [128, 128]
    return {
        "wb": wb.astype(bf16),
        "u8": u8.astype(fp8),
        "ud8": ud8.astype(fp8),
        "bdec8": bdec8.astype(fp8),
        "ones8": np.ones((1, 2 * SB), dtype=fp8),
        "wdd2": wdd2.astype(bf16),
        "bdrow": bd.reshape(1, -1).astype(bf16),
        "ones": np.ones((1, 128), dtype=bf16),
        "ident": np.eye(128, dtype=np.float32),
    }


def kernel(inputs, W, U, b, Wd, bd, out_steps):
    inputs = np.asarray(inputs, dtype=np.float32)
    W = np.asarray(W, dtype=np.float32)
    U_ = np.asarray(U, dtype=np.float32)
    b_ = np.asarray(b, dtype=np.float32)
    Wd = np.asarray(Wd, dtype=np.float32)
    bd = np.asarray(bd, dtype=np.float32)
    out_steps = int(out_steps)

    B_full, T, _ = inputs.shape
    assert B_full % N_CORES == 0
    Bc = B_full // N_CORES

    nc = _get_program((Bc, T, out_steps))
    shared = _host_prep(W, U_, b_, Wd, bd)
    in_maps = [
        {"xin": np.ascontiguousarray(inputs[i * Bc : (i + 1) * Bc]), **shared}
        for i in range(N_CORES)
    ]
    res = bass_utils.run_bass_kernel_spmd(nc, in_maps, core_ids=list(range(N_CORES)))
    out = np.concatenate([res.results[i]["yout"] for i in range(N_CORES)], axis=0)
    return out


# revision 11
# speedup vs baseline: 1.1121x; 1.1002x over previous
"""LSTM warmup + autoregressive decode kernel for Trainium2 (Bass/Tile).

Reference computation (per batch row):
  h,c = 0
  for t in range(T):  h,c = LSTMstep(x_t)        # warmup over input seq
  pred0 = h @ Wd + bd
  for d in range(out_steps-1): h,c = LSTMstep(pred_d); pred_{d+1} = h@Wd+bd
  out[b, s, f] = pred_s

Strategy: data-parallel over 8 NeuronCores (B=4096 -> 512/core); the 512-row
shard splits into FOUR interleaved 128-row streams so each stream's serial
recurrence tail (tanh(c) -> h8 -> h@U matmuls) hides under the other three
streams' activation work.

Key optimizations over the bf16 per-gate 2-stream baseline (Act engine ~98%
busy was the bottleneck):
 - All-sigmoid gates: the g-gate tanh is computed as 2*sigmoid(2 z_g) - 1
   (g-gate weight columns pre-scaled x2 on the host), so all four gates of a
   stream activate in ONE [128, 1024] Act op reading the 2-bank PSUM z tile,
   amortizing the ~185ns per-op Act overhead.
 - fp8e4m3 DoubleRow recurrence: h and U/Ud are fp8; each h-matmul packs the
   two 128-unit K-tiles into one DoubleRow matmul, ~4x cheaper on PE than
   two bf16 K=128 passes.  The warmup x-pass and pred path stay bf16 (fp8
   there fails the 2e-2 tolerance; validated in numpy).
 - Decode bias via a DoubleRow ones-matmul (K=1 x 2 k-tiles, second zero).
 - tanh(c) on Act, PAIRED across streams (c tiles of streams 0+1 / 2+3 share
   one [128, 512] tile activated in one op) to halve tc op overheads.
 - PSUM: 4 streams x 2 banks = all 8 banks.  x-transpose scratch (warmup)
   and pred accumulators (decode) borrow stream 3's second bank during its
   dead window (after g3's activation read, before the next step's matmuls);
   PSUM pending-zero semantics make the multi-use safe.
 - Within a z bank the accumulation group is bracketed once per step: the
   h-independent x/bias passes issue early (first marks the zero region),
   the h-DR passes close the group once h8 lands, overlapping PE x-work
   with the previous step's Act/DVE tail.
 - Elementwise h work that tolerates latency runs on the idle engines:
   x-staging copy and the bf16 h for the pred path go to GPSIMD.
"""

import sys

for _p in ("/opt/trn_rl_repo", "/root/.axon_site/_ro/trn_rl_repo"):
    if _p not in sys.path:
        sys.path.insert(0, _p)

import numpy as np

import concourse.bacc as bacc
import concourse.mybir as mybir
import concourse.tile as tile
from concourse import bass_utils

F32 = mybir.dt.float32
BF16 = mybir.dt.bfloat16
FP8 = mybir.dt.float8e4
AF = mybir.ActivationFunctionType
ALU = mybir.AluOpType
DR = mybir.MatmulPerfMode.DoubleRow

N_CORES = 8
F = 64          # input/output feature dim
U = 256         # lstm units
U4 = 4 * U      # gate rows
XLOOK = 3       # steps of x-staging lookahead
NXS = 6         # static x tiles
NS = 4          # batch streams per core
SB = 128        # stream batch

# gate order inside a stream's z tile (cols: gate, u-chunk, batch)
G_I, G_F, G_G, G_O = 0, 1, 2, 3
GCOL = {G_I: 0, G_F: 256, G_G: 512, G_O: 768}
ZW = 4 * 2 * SB          # z tile width per stream (1024 f32 = 2 banks)
AUXC = 2 * 2 * SB        # col base of stream-3's second bank (scratch window)


def build_program(B, T, out_steps):
    """Single-core SPMD program for a batch shard of size B (=512)."""
    assert B == 512, "tile geometry is hardcoded for a 512-row shard"
    NB = B // 128

    nc = bacc.Bacc("TRN2", target_bir_lowering=False, debug=False, num_devices=1)

    xin = nc.dram_tensor("xin", [B, T, F], F32, kind="ExternalInput").ap()
    wb_d = nc.dram_tensor("wb", [F + 1, U4], BF16, kind="ExternalInput").ap()
    u8_d = nc.dram_tensor("u8", [128, 2 * U4], FP8, kind="ExternalInput").ap()
    ud8_d = nc.dram_tensor("ud8", [128, 2 * U4], FP8, kind="ExternalInput").ap()
    bdec8_d = nc.dram_tensor("bdec8", [1, 2 * U4], FP8, kind="ExternalInput").ap()
    ones8_d = nc.dram_tensor("ones8", [1, 2 * SB], FP8, kind="ExternalInput").ap()
    wdd2_d = nc.dram_tensor("wdd2", [128, 2 * F], BF16, kind="ExternalInput").ap()
    bdrow_d = nc.dram_tensor("bdrow", [1, F], BF16, kind="ExternalInput").ap()
    ones_d = nc.dram_tensor("ones", [1, 128], BF16, kind="ExternalInput").ap()
    ident_d = nc.dram_tensor("ident", [128, 128], F32, kind="ExternalInput").ap()
    yout = nc.dram_tensor("yout", [B, out_steps, F], F32, kind="ExternalOutput").ap()

    xin_c = xin.rearrange("(c p) t f -> p c (t f)", c=4)   # [128, 4, T*F]
    yout_c = yout.rearrange("(c p) s f -> p c (s f)", c=4)  # [128, 4, S*F]

    with tile.TileContext(nc) as tc:
        import contextlib

        with contextlib.ExitStack() as ctx:
            wpool = ctx.enter_context(tc.tile_pool(name="wpool", bufs=1))
            xspool = ctx.enter_context(tc.tile_pool(name="xspool", bufs=1))
            dpool = ctx.enter_context(tc.tile_pool(name="dpool", bufs=16))
            gpool = ctx.enter_context(tc.tile_pool(name="gpool", bufs=2))
            tpool = ctx.enter_context(tc.tile_pool(name="tpool", bufs=2))
            cpool = ctx.enter_context(tc.tile_pool(name="cpool", bufs=2))
            tcpool = ctx.enter_context(tc.tile_pool(name="tcpool", bufs=2))
            h8pool = ctx.enter_context(tc.tile_pool(name="h8pool", bufs=3))
            hbpool = ctx.enter_context(tc.tile_pool(name="hbpool", bufs=3))
            opool = ctx.enter_context(tc.tile_pool(name="opool", bufs=4))
            zpool = ctx.enter_context(tc.tile_pool(name="zpool", bufs=1, space="PSUM"))

            # ---- weights / constants ----
            ident = wpool.tile([128, 128], F32)
            wb = wpool.tile([F + 1, U4], BF16)
            u8 = wpool.tile([128, 2 * U4], FP8)
            ud8 = wpool.tile([128, 2 * U4], FP8)
            bdec8 = wpool.tile([1, 2 * U4], FP8)
            ones8 = wpool.tile([1, 2 * SB], FP8)
            wdd2 = wpool.tile([128, 2 * F], BF16)
            bdrow = wpool.tile([1, F], BF16)
            ones = wpool.tile([1, 128], BF16)

            # static x tiles: rows 0:64 = x_t^T (bf16), row 64 = ones
            xs = [xspool.tile([F + 1, B], BF16, name=f"xs{j}") for j in range(NXS)]
            for j in range(NXS):
                nc.gpsimd.memset(xs[j][F : F + 1, :], 1.0)

            # per-stream z PSUM tiles: [128, 1024] f32 = 2 banks each
            zt = [zpool.tile([128, ZW], F32, name=f"z{st}") for st in range(NS)]

            # PE warm-up through the startup DMA wait (p-state ramp)
            for j in range(3):
                nc.tensor.matmul(
                    zt[0][:, 0:512], xs[0][F : F + 1, 0:128], xs[0][F : F + 1, 0:512],
                    start=True, stop=True, skip_group_check=True,
                )

            # ---- x staging ----
            def stage_x_dma(t):
                dt_in = dpool.tile([128, NB * F], F32, tag="din", name=f"din{t}")
                nc.sync.dma_start(
                    dt_in[:].rearrange("p (c f) -> p c f", c=NB),
                    xin_c[:, :, F * t : F * (t + 1)],
                )
                return dt_in

            def stage_x_transpose(t, dt_in):
                # transpose into STREAM 0's second bank: its dead window opens
                # right after g0 (the first Act op of the step), so the DVE
                # copy executes early in the period where DVE has slack --
                # keeping it off the last stream's chain critical path.
                # (Pool/GPSIMD cannot read PSUM on real HW; copy on DVE.)
                xp = zt[0][0:F, AUXC : AUXC + B]
                for bc in range(NB):
                    nc.tensor.transpose(
                        xp[:, 128 * bc : 128 * (bc + 1)],
                        dt_in[:, F * bc : F * (bc + 1)], ident[:],
                    )
                nc.vector.tensor_copy(xs[t % NXS][0:F, :], xp[:, :])

            u8_2 = u8.rearrange("p (two m) -> p two m", two=2)
            ud8_2 = ud8.rearrange("p (two m) -> p two m", two=2)
            bdec8_2 = bdec8.rearrange("p (two m) -> p two m", two=2)
            ones8_2 = ones8.rearrange("p (two n) -> p two n", two=2)

            # ---- PE z-block emission (bank = 2 gates; one group per bank) --
            def emit_x_passes(st, x_t, close):
                """h-independent z work: x@W+b (warmup) or bias (decode)."""
                z = zt[st]
                for q in (G_I, G_F, G_G, G_O):
                    for ch in (0, 1):
                        dst = z[:, q * 2 * SB + ch * SB
                                : q * 2 * SB + (ch + 1) * SB]
                        mcol = GCOL[q] + 128 * ch
                        bank_first = q in (G_I, G_G) and ch == 0
                        bank_last = q in (G_F, G_O) and ch == 1
                        if x_t is not None:
                            nc.tensor.matmul(
                                dst, wb[:, mcol : mcol + 128],
                                x_t[:, SB * st : SB * (st + 1)],
                                start=bank_first, stop=close and bank_last,
                                skip_group_check=True,
                            )
                        else:
                            nc.tensor.matmul(
                                dst, bdec8_2[:, :, mcol : mcol + 128],
                                ones8_2[:, :, 0:SB],
                                start=bank_first, stop=close and bank_last,
                                perf_mode=DR, skip_group_check=True,
                            )

            def emit_h_passes(st, uw2, h8_ap):
                """DoubleRow h@U accumulation closing each gate-bank group."""
                z = zt[st]
                h2 = h8_ap.rearrange("p (two n) -> p two n", two=2)
                for q in (G_I, G_F, G_G, G_O):
                    for ch in (0, 1):
                        dst = z[:, q * 2 * SB + ch * SB
                                : q * 2 * SB + (ch + 1) * SB]
                        mcol = GCOL[q] + 128 * ch
                        bank_last = q in (G_F, G_O) and ch == 1
                        nc.tensor.matmul(
                            dst, uw2[:, :, mcol : mcol + 128], h2[:],
                            start=False, stop=bank_last,
                            perf_mode=DR, skip_group_check=True,
                        )

            # ---- Act / DVE ----
            def gslice(g_t, q):
                return g_t[:, q * 2 * SB : (q + 1) * 2 * SB]

            def emit_gate_act(t, st):
                g_t = gpool.tile([128, ZW], BF16, tag=f"g{st}", name=f"g{t}_{st}")
                nc.scalar.activation(g_t[:], zt[st][:], AF.Sigmoid)
                return g_t

            def emit_chain(t, st, g_t, c_pair, c_prev):
                """c = f*c_prev + i*(2*sg - 1) into half of the shared pair
                tile; sg = sigmoid(2 z_g)."""
                c_t = c_pair[:, 2 * SB * (st % 2) : 2 * SB * (st % 2 + 1)]
                m2 = tpool.tile([128, 2 * SB], BF16, tag=f"m{st}",
                                name=f"m{t}_{st}")
                i_s, f_s = gslice(g_t, G_I), gslice(g_t, G_F)
                sg_s = gslice(g_t, G_G)
                if c_prev is None:
                    nc.vector.tensor_mul(m2[:], i_s, sg_s)
                    nc.vector.scalar_tensor_tensor(
                        c_t, m2[:], 2.0, i_s, ALU.mult, ALU.subtract)
                else:
                    fc = tpool.tile([128, 2 * SB], BF16, tag=f"fc{st}",
                                    name=f"fc{t}_{st}")
                    t1 = tpool.tile([128, 2 * SB], BF16, tag=f"t{st}",
                                    name=f"t{t}_{st}")
                    # fc on the idle GPSIMD: runs concurrently with this
                    # stream's m2/t1 on DVE, shrinking the DVE queue
                    nc.gpsimd.tensor_mul(fc[:], f_s, c_prev)
                    nc.vector.tensor_mul(m2[:], i_s, sg_s)
                    nc.vector.scalar_tensor_tensor(
                        t1[:], m2[:], 2.0, i_s, ALU.mult, ALU.subtract)
                    nc.vector.tensor_add(c_t, fc[:], t1[:])
                return c_t

            # ---- pred + output (stream-0 second-bank dead window) ----
            def emit_pred_mm(st, h_t):
                pp = zt[0][:, AUXC + 2 * F * st : AUXC + 2 * F * st + F]
                nc.tensor.matmul(
                    pp, ones[0:1, 0:128], bdrow[:], start=True, stop=False,
                    skip_group_check=True,
                )
                nc.tensor.matmul(
                    pp, h_t[:, 0:128], wdd2[:, 0:F],
                    start=False, stop=False, skip_group_check=True,
                )
                nc.tensor.matmul(
                    pp, h_t[:, SB : SB + 128], wdd2[:, F : 2 * F],
                    start=False, stop=True, skip_group_check=True,
                )
                return pp

            def emit_pred_out(s, st, pp):
                osb = opool.tile([128, F], F32, tag="ot", name=f"o{s}_{st}")
                nc.vector.tensor_copy(osb[:], pp)
                nc.sync.dma_start(
                    yout_c[:, st : st + 1, F * s : F * (s + 1)],
                    osb[:].rearrange("p (c f) -> p c f", c=1),
                )

            # ---- prologue ----
            dma_q = {}
            dma_q[0] = stage_x_dma(0)
            nc.sync.dma_start(ident[:], ident_d[:])
            nc.sync.dma_start(wb[:], wb_d[:])
            for t in range(1, min(XLOOK, T)):
                dma_q[t] = stage_x_dma(t)
            nc.sync.dma_start(u8[:], u8_d[:])
            nc.sync.dma_start(ones8[:], ones8_d[:])
            nc.sync.dma_start(ones[:], ones_d[:])
            nc.sync.dma_start(ud8[:], ud8_d[:])
            nc.sync.dma_start(wdd2[:], wdd2_d[:])
            nc.sync.dma_start(bdec8[:], bdec8_d[:])
            nc.sync.dma_start(bdrow[:], bdrow_d[:])
            for t in range(min(XLOOK, T)):
                stage_x_transpose(t, dma_q.pop(t))
            if XLOOK < T:
                dma_q[XLOOK] = stage_x_dma(XLOOK)

            n_steps = T + (out_steps - 1)
            h8_prev = [None] * NS   # (tile, col offset) per stream
            c_prev = [None] * NS
            hbs = {}

            for t in range(n_steps):
                warm = t < T
                x_t = xs[t % NXS] if warm else None
                uw2 = u8_2 if warm else ud8_2
                first = t == 0
                need_hb = t >= T - 1

                # PE: h-independent passes open each bank group; DR h passes
                # close them (waiting on h8 of t-1); then the merged gate act
                g_ts = []
                for st in range(NS):
                    emit_x_passes(st, x_t, close=first)
                    if not first:
                        emit_h_passes(st, uw2, h8_prev[st])
                    g_ts.append(emit_gate_act(t, st))

                # DVE chains into shared pair tiles; paired tanh(c) on Act
                c01 = cpool.tile([128, 4 * SB], BF16, tag="c01", name=f"c01_{t}")
                c23 = cpool.tile([128, 4 * SB], BF16, tag="c23", name=f"c23_{t}")
                pair = {0: c01, 1: c01, 2: c23, 3: c23}
                c_ts = [emit_chain(t, st, g_ts[st], pair[st], c_prev[st])
                        for st in range(NS)]
                c_prev = c_ts

                tc01 = tcpool.tile([128, 4 * SB], BF16, tag="tc01",
                                   name=f"tc01_{t}")
                tc23 = tcpool.tile([128, 4 * SB], BF16, tag="tc23",
                                   name=f"tc23_{t}")
                nc.scalar.activation(tc01[:], c01[:], AF.Tanh)
                nc.scalar.activation(tc23[:], c23[:], AF.Tanh)
                tcp = {0: tc01, 1: tc01, 2: tc23, 3: tc23}

                # h8 (fp8 for the DR recurrence) on DVE; bf16 h on GPSIMD
                h8p01 = h8pool.tile([128, 4 * SB], FP8, tag="h801",
                                    name=f"h801_{t}")
                h8p23 = h8pool.tile([128, 4 * SB], FP8, tag="h823",
                                    name=f"h823_{t}")
                h8p = {0: h8p01, 1: h8p01, 2: h8p23, 3: h8p23}
                for st in range(NS):
                    off = 2 * SB * (st % 2)
                    nc.vector.tensor_mul(
                        h8p[st][:, off : off + 2 * SB], gslice(g_ts[st], G_O),
                        tcp[st][:, off : off + 2 * SB],
                    )
                    h8_prev[st] = h8p[st][:, off : off + 2 * SB]
                if need_hb:
                    hbt = hbpool.tile([128, NS * 2 * SB], BF16, tag="hb",
                                      name=f"hb_{t}")
                    for st in range(NS):
                        off = 2 * SB * (st % 2)
                        nc.gpsimd.tensor_mul(
                            hbt[:, 2 * SB * st : 2 * SB * (st + 1)],
                            gslice(g_ts[st], G_O),
                            tcp[st][:, off : off + 2 * SB],
                        )
                    hbs[t] = hbt

                # x staging for upcoming steps (stream-3 second-bank window)
                if t + 1 < n_steps:
                    if t + XLOOK < T and t + XLOOK in dma_q:
                        stage_x_transpose(t + XLOOK, dma_q.pop(t + XLOOK))
                    if t + XLOOK + 1 < T:
                        dma_q[t + XLOOK + 1] = stage_x_dma(t + XLOOK + 1)

                # pred matmuls for the previous step's h (lag 1)
                if t >= T and (t - 1) in hbs:
                    hbt = hbs.pop(t - 1)
                    for st in range(NS):
                        pp = emit_pred_mm(st, hbt[:, 2 * SB * st : 2 * SB * (st + 1)])
                        emit_pred_out(t - T, st, pp)

            # epilogue: final preds
            hbt = hbs.pop(n_steps - 1)
            for st in range(NS):
                pp = emit_pred_mm(st, hbt[:, 2 * SB * st : 2 * SB * (st + 1)])
                emit_pred_out(out_steps - 1, st, pp)

    nc.compile()
    return nc


_CACHE = {}


def _get_program(key):
    if key not in _CACHE:
        _CACHE[key] = build_program(*key)
    return _CACHE[key]


def _host_prep(W, Uk, b, Wd, bd):
    bf16 = mybir.dt.np(BF16)
    fp8 = mybir.dt.np(FP8)
    W64 = W.astype(np.float64)
    Ud = (Uk.astype(np.float64) + Wd.astype(np.float64) @ W64).astype(np.float32)
    bdec = (b.astype(np.float64) + bd.astype(np.float64) @ W64).astype(np.float32)

    def scale_g(M):
        M = M.copy()
        M[..., 512:768] *= 2.0  # g-gate cols: tanh(z) = 2*sigmoid(2z) - 1
        return M

    Ws, bs = scale_g(W), scale_g(b)
    Us, Uds, bdecs = scale_g(Uk), scale_g(Ud), scale_g(bdec)
    wb = np.concatenate([Ws, bs.reshape(1, -1)], axis=0)          # [65, 1024]
    u8 = np.concatenate([Us[0:128], Us[128:256]], axis=1)         # [128, 2048]
    ud8 = np.concatenate([Uds[0:128], Uds[128:256]], axis=1)
    bdec8 = np.concatenate(
        [bdecs.reshape(1, -1), np.zeros((1, U4), np.float32)], axis=1)
    wdd2 = np.concatenate([Wd[0:128], Wd[128:256]], axis=1)       # [128, 128]
    return {
        "wb": wb.astype(bf16),
        "u8": u8.astype(fp8),
        "ud8": ud8.astype(fp8),
        "bdec8": bdec8.astype(fp8),
        "ones8": np.ones((1, 2 * SB), dtype=fp8),
        "wdd2": wdd2.astype(bf16),
        "bdrow": bd.reshape(1, -1).astype(bf16),
        "ones": np.ones((1, 128), dtype=bf16),
        "ident": np.eye(128, dtype=np.float32),
    }


def kernel(inputs, W, U, b, Wd, bd, out_steps):
    inputs = np.asarray(inputs, dtype=np.float32)
    W = np.asarray(W, dtype=np.float32)
    U_ = np.asarray(U, dtype=np.float32)
    b_ = np.asarray(b, dtype=np.float32)
    Wd = np.asarray(Wd, dtype=np.float32)
    bd = np.asarray(bd, dtype=np.float32)
    out_steps = int(out_steps)

    B_full, T, _ = inputs.shape
    assert B_full % N_CORES == 0
    Bc = B_full // N_CORES

    nc = _get_program((Bc, T, out_steps))
    shared = _host_prep(W, U_, b_, Wd, bd)
    in_maps = [
        {"xin": np.ascontiguousarray(inputs[i * Bc : (i + 1) * Bc]), **shared}
        for i in range(N_CORES)
    ]
    res = bass_utils.run_bass_kernel_spmd(nc, in_maps, core_ids=list(range(N_CORES)))
    out = np.concatenate([res.results[i]["yout"] for i in range(N_CORES)], axis=0)
    return out
